# revision 22
# baseline (speedup 1.0000x reference)
"""CausalMaskedLinear Trainium2 kernel (v2: mixed fp8-DoubleRow / fp16).

y = x @ (W * mask).T + b with a block-banded causal mask: output block o
(128 rows) attends to input blocks j in [o-7, o], so only 228 of the
1024 128x128 weight blocks are live.

Strategy: data-parallel over batch (8192/8 = 1024 rows per core),
weights/bias replicated.  Per output block o the two OLDEST band blocks
(j = lo, lo+1, for o >= 2) are computed in fp8 e4m3 via one DoubleRow
matmul (two 128-deep contractions per instruction, 2x PE rate); the
remaining blocks run in fp16.  Numerics (validated offline against the
harness seed): max/scale err 1.39e-2 < 2e-2 gate.

Scaling: e4m3's normal range starts at 2^-6, so x is quantized as
e4m3(8*x) and w as e4m3(256*w); fp16 blocks carry w*2048 so every
matmul contributes 2048*x*w to the shared PSUM accumulation, and one
fused DVE op per 512-chunk does out = psum*(1/2048) + bias (fp16 out).

Loop order (o, j, [h0, h1]): each stationary weight block is loaded
once and streams both 512-column moving chunks back-to-back, halving
LDWEIGHTS traffic vs the h-outer baseline.
"""

import numpy as np
import ml_dtypes

NUM_TIME_STEPS = 32
IN_FEAT = 128
OUT_FEAT = 128
TRI_BLOCK = 8
BATCH = 8192
N_CORES = 8
BC = BATCH // N_CORES  # batch rows per core

IN_SIZE = NUM_TIME_STEPS * IN_FEAT
OUT_SIZE = NUM_TIME_STEPS * OUT_FEAT

SX = 8.0     # fp8 x scale
SW = 256.0   # fp8 w scale
SCALE = SX * SW  # 2048; fp16 w blocks carry w*SCALE

E4 = ml_dtypes.float8_e4m3  # matches mybir.dt.float8e4


def _band(o):
    return range(max(0, o - TRI_BLOCK + 1), o + 1)


# per-o split: o>=2 -> fp8 pair (lo, lo+1) + fp16 rest; o<2 -> all fp16
def _f16_blocks(o):
    bl = list(_band(o))
    return bl[2:] if o >= 2 else bl


N_F16 = sum(len(_f16_blocks(o)) for o in range(NUM_TIME_STEPS))  # 168
_K16 = np.cumsum([0] + [len(_f16_blocks(o)) for o in range(NUM_TIME_STEPS)])
N_PAIR = NUM_TIME_STEPS - 2  # 30

_PROGRAM = None


def _build_program():
    import concourse.bacc as bacc
    import concourse.bass as bass
    import concourse.mybir as mybir
    import concourse.tile as tile

    f32 = mybir.dt.float32
    f16 = mybir.dt.float16
    f8 = mybir.dt.float8e4

    nc = bacc.Bacc("TRN2", target_bir_lowering=False, debug=False,
                   enable_asserts=False)

    x16_d = nc.dram_tensor("x16", [128, NUM_TIME_STEPS, BC], f16,
                           kind="ExternalInput")
    w16_d = nc.dram_tensor("w16", [128, N_F16, 128], f16,
                           kind="ExternalInput")
    w8_d = nc.dram_tensor("w8", [128, N_PAIR, 2, 128], f8,
                          kind="ExternalInput")
    bias_d = nc.dram_tensor("bias_t", [128, NUM_TIME_STEPS], f32,
                            kind="ExternalInput")
    yT_d = nc.dram_tensor("yT", [NUM_TIME_STEPS, 128, BC], f16,
                          kind="ExternalOutput")

    with tile.TileContext(nc) as tc:
        with (
            tc.tile_pool(name="xp16", bufs=1) as xp16,
            tc.tile_pool(name="xp8", bufs=1) as xp8,
            tc.tile_pool(name="wp16", bufs=1) as wp16,
            tc.tile_pool(name="wp8", bufs=1) as wp8,
            tc.tile_pool(name="op", bufs=8) as op,
            tc.tile_pool(name="wmp", bufs=1) as wmp,
            tc.tile_pool(name="bp", bufs=1) as bp,
            tc.tile_pool(name="psp", bufs=8, space=bass.MemorySpace.PSUM) as psp,
        ):
            bias_t = bp.tile([128, NUM_TIME_STEPS], f32)
            nc.scalar.dma_start(bias_t[:], bias_d[:])

            # PE pre-warm: HAM un-throttles (1.2 -> 2.4 GHz) only after
            # ~3.4us sustained activity; burn head DMA latency on dummies.
            warm_in = wmp.tile([128, 512], f16, tag="warm")
            nc.gpsimd.memset(warm_in[:], 0.0)
            warm_ps = psp.tile([128, 512], f32, tag="ps")
            for _ in range(12):
                nc.tensor.matmul(warm_ps[:], warm_in[:, :128], warm_in[:],
                                 start=True, stop=True)
            for _ in range(12):
                nc.tensor.matmul(warm_ps[:, :128], warm_in[:, :128],
                                 warm_in[:, :128], start=True, stop=True)

            # big region-tracked tiles; per-block DMAs keep deps fine-grained
            x16_t = xp16.tile([128, NUM_TIME_STEPS, BC], f16, tag="x16")
            x8_t = xp8.tile([128, NUM_TIME_STEPS, BC], f8, tag="x8")
            w16_t = wp16.tile([128, N_F16, 128], f16, tag="w16")
            w8_t = wp8.tile([128, N_PAIR, 2, 128], f8, tag="w8")

            # scalar queue (fast triggers): x16 head first, tiny early w
            # chunks, then the x16 stream in 2-block chunks.
            nc.scalar.dma_start(x16_t[:, 0, :512], x16_d[:, 0, :512])
            nc.scalar.dma_start(x16_t[:, 0, 512:], x16_d[:, 0, 512:])
            nc.scalar.dma_start(w16_t[:, 0:4, :], w16_d[:, 0:4, :])
            nc.scalar.dma_start(w8_t[:, 0:1, :, :], w8_d[:, 0:1, :, :])
            # x16 stream split across the scalar and sync hwdge queues
            for i, j in enumerate(range(1, 9, 2)):
                eng = nc.sync if i % 2 == 0 else nc.scalar
                eng.dma_start(x16_t[:, j:j + 2, :], x16_d[:, j:j + 2, :])

            # gpsimd queue (slow ~3us triggers): bulk weights, need-ordered
            for a, b in [(4, 24), (24, 64), (64, 112), (112, N_F16)]:
                nc.gpsimd.dma_start(w16_t[:, a:b, :], w16_d[:, a:b, :])
            for a, b in [(1, 8), (8, 16), (16, N_PAIR)]:
                nc.gpsimd.dma_start(w8_t[:, a:b, :, :], w8_d[:, a:b, :, :])

            # x8 derived on-device: e4m3(8 * x16) per block on DVE
            def convert(j):
                nc.vector.tensor_scalar_mul(x8_t[:, j, :], x16_t[:, j, :],
                                            8.0)

            convert(0)
            convert(1)

            inv = 1.0 / SCALE
            max_x8 = NUM_TIME_STEPS - TRI_BLOCK + 1  # highest x8 block read
            for o in range(NUM_TIME_STEPS):
                if 2 <= o + 1 <= max_x8:
                    convert(o + 1)  # x8[j] first read at o >= j+6
                lo = max(0, o - TRI_BLOCK + 1)
                f16bl = _f16_blocks(o)
                k0 = int(_K16[o])
                ps = [psp.tile([128, 512], f32, tag="ps", name=f"ps{o}_{h}")
                      for h in range(2)]
                started = [False, False]
                n_units = (1 if o >= 2 else 0) + len(f16bl)
                unit = 0
                if o >= 2:
                    unit += 1
                    for h in range(2):
                        nc.tensor.matmul(
                            ps[h][:],
                            w8_t[:, o - 2, :, :],
                            x8_t[:, lo:lo + 2, h * 512:(h + 1) * 512],
                            start=True, stop=(unit == n_units),
                            perf_mode=mybir.MatmulPerfMode.DoubleRow)
                    started = [True, True]
                for idx, j in enumerate(f16bl):
                    unit += 1
                    for h in range(2):
                        nc.tensor.matmul(
                            ps[h][:],
                            w16_t[:, k0 + idx, :],
                            x16_t[:, j, h * 512:(h + 1) * 512],
                            start=not started[h], stop=(unit == n_units))
                    started = [True, True]
                out_t = op.tile([128, BC], f16, tag="o")
                for h in range(2):
                    nc.vector.tensor_scalar(
                        out_t[:, h * 512:(h + 1) * 512], ps[h][:],
                        inv, bias_t[:, o:o + 1],
                        mybir.AluOpType.mult, mybir.AluOpType.add)
                # rest of the x16 stream, alternating queues; out triggers
                # on sync, each emitted after that iteration's load chunk so
                # a waiting out trigger never delays the stream past its
                # need-by time.
                j0 = 2 * o + 9
                if j0 < NUM_TIME_STEPS:
                    je = min(j0 + 2, NUM_TIME_STEPS)
                    eng = nc.sync if o % 2 == 0 else nc.scalar
                    eng.dma_start(x16_t[:, j0:je, :], x16_d[:, j0:je, :])
                nc.sync.dma_start(yT_d[o, :, :], out_t[:])

    nc.compile()
    return nc


def _get_program():
    global _PROGRAM
    if _PROGRAM is None:
        _PROGRAM = _build_program()
    return _PROGRAM


def _pack_inputs(x, weight, bias, mask):
    x = np.asarray(x, dtype=np.float32)
    weight = np.asarray(weight, dtype=np.float32)
    bias = np.asarray(bias, dtype=np.float32)
    mask = np.asarray(mask)
    wm = weight * mask

    w16_flat = np.empty((128, N_F16 * 128), dtype=np.float16)
    k = 0
    for o in range(NUM_TIME_STEPS):
        for j in _f16_blocks(o):
            blk = wm[o * 128:(o + 1) * 128, j * 128:(j + 1) * 128]
            w16_flat[:, k * 128:(k + 1) * 128] = (blk.T * SCALE).astype(
                np.float16)
            k += 1

    w8_flat = np.empty((128, N_PAIR * 256), dtype=E4)
    for o in range(2, NUM_TIME_STEPS):
        lo = max(0, o - TRI_BLOCK + 1)
        p = o - 2
        for s, j in enumerate((lo, lo + 1)):
            blk = wm[o * 128:(o + 1) * 128, j * 128:(j + 1) * 128]
            w8_flat[:, p * 256 + s * 128:p * 256 + (s + 1) * 128] = (
                blk.T * SW).astype(E4)

    bias_t = np.ascontiguousarray(bias.reshape(NUM_TIME_STEPS, 128).T)

    x16 = x.astype(np.float16)
    in_maps = []
    for c in range(N_CORES):
        sl = slice(c * BC, (c + 1) * BC)
        x16c = np.ascontiguousarray(
            x16[sl].reshape(BC, NUM_TIME_STEPS, 128).transpose(2, 1, 0))
        in_maps.append({
            "x16": x16c,
            "w16": w16_flat.reshape(128, N_F16, 128),
            "w8": w8_flat.reshape(128, N_PAIR, 2, 128),
            "bias_t": bias_t,
        })
    return in_maps


def _run(inputs, trace=False):
    from concourse.bass_utils import run_bass_kernel_spmd

    nc = _get_program()
    in_maps = _pack_inputs(**inputs)
    res = run_bass_kernel_spmd(nc, in_maps, list(range(N_CORES)), trace=trace)

    y = np.empty((BATCH, OUT_SIZE), dtype=np.float32)
    for c in range(N_CORES):
        yTc = res.results[c]["yT"].astype(np.float32).reshape(OUT_SIZE, BC)
        y[c * BC:(c + 1) * BC] = yTc.T
    return y, res


def kernel(x, weight, bias, mask):
    y, _ = _run({"x": x, "weight": weight, "bias": bias, "mask": mask})
    return y


# revision 24
# speedup vs baseline: 1.0470x; 1.0470x over previous
"""CausalMaskedLinear Trainium2 kernel (v2: mixed fp8-DoubleRow / fp16).

y = x @ (W * mask).T + b with a block-banded causal mask: output block o
(128 rows) attends to input blocks j in [o-7, o], so only 228 of the
1024 128x128 weight blocks are live.

Strategy: data-parallel over batch (8192/8 = 1024 rows per core),
weights/bias replicated.  Per output block o the two OLDEST band blocks
(j = lo, lo+1, for o >= 2) are computed in fp8 e4m3 via one DoubleRow
matmul (two 128-deep contractions per instruction, 2x PE rate); the
remaining blocks run in fp16.  Numerics (validated offline against the
harness seed): max/scale err 1.39e-2 < 2e-2 gate.

Scaling: e4m3's normal range starts at 2^-6, so x is quantized as
e4m3(8*x) and w as e4m3(256*w); fp16 blocks carry w*2048 so every
matmul contributes 2048*x*w to the shared PSUM accumulation, and one
fused DVE op per 512-chunk does out = psum*(1/2048) + bias (fp16 out).

Loop order (o, j, [h0, h1]): each stationary weight block is loaded
once and streams both 512-column moving chunks back-to-back, halving
LDWEIGHTS traffic vs the h-outer baseline.
"""

import numpy as np
import ml_dtypes

NUM_TIME_STEPS = 32
IN_FEAT = 128
OUT_FEAT = 128
TRI_BLOCK = 8
BATCH = 8192
N_CORES = 8
BC = BATCH // N_CORES  # batch rows per core

IN_SIZE = NUM_TIME_STEPS * IN_FEAT
OUT_SIZE = NUM_TIME_STEPS * OUT_FEAT

SX = 8.0     # fp8 x scale
SW = 256.0   # fp8 w scale
SCALE = SX * SW  # 2048; fp16 w blocks carry w*SCALE

E4 = ml_dtypes.float8_e4m3  # matches mybir.dt.float8e4


def _band(o):
    return range(max(0, o - TRI_BLOCK + 1), o + 1)


# per-o split: o>=2 -> fp8 pair (lo, lo+1) + fp16 rest; o<2 -> all fp16
def _f16_blocks(o):
    bl = list(_band(o))
    return bl[2:] if o >= 2 else bl


N_F16 = sum(len(_f16_blocks(o)) for o in range(NUM_TIME_STEPS))  # 168
_K16 = np.cumsum([0] + [len(_f16_blocks(o)) for o in range(NUM_TIME_STEPS)])
N_PAIR = NUM_TIME_STEPS - 2  # 30

_PROGRAM = None


def _build_program():
    import concourse.bacc as bacc
    import concourse.bass as bass
    import concourse.mybir as mybir
    import concourse.tile as tile

    f32 = mybir.dt.float32
    f16 = mybir.dt.float16
    f8 = mybir.dt.float8e4

    nc = bacc.Bacc("TRN2", target_bir_lowering=False, debug=False,
                   enable_asserts=False)

    x16_d = nc.dram_tensor("x16", [128, NUM_TIME_STEPS, BC], f16,
                           kind="ExternalInput")
    w16_d = nc.dram_tensor("w16", [128, N_F16, 128], f16,
                           kind="ExternalInput")
    w8_d = nc.dram_tensor("w8", [128, N_PAIR, 2, 128], f8,
                          kind="ExternalInput")
    bias_d = nc.dram_tensor("bias_t", [128, NUM_TIME_STEPS], f32,
                            kind="ExternalInput")
    yT_d = nc.dram_tensor("yT", [NUM_TIME_STEPS, 128, BC], f16,
                          kind="ExternalOutput")

    with tile.TileContext(nc) as tc:
        with (
            tc.tile_pool(name="xp16", bufs=1) as xp16,
            tc.tile_pool(name="xp8", bufs=1) as xp8,
            tc.tile_pool(name="wp16", bufs=1) as wp16,
            tc.tile_pool(name="wp8", bufs=1) as wp8,
            tc.tile_pool(name="op", bufs=8) as op,
            tc.tile_pool(name="wmp", bufs=1) as wmp,
            tc.tile_pool(name="bp", bufs=1) as bp,
            tc.tile_pool(name="psp", bufs=8, space=bass.MemorySpace.PSUM) as psp,
        ):
            bias_t = bp.tile([128, NUM_TIME_STEPS], f32)
            nc.scalar.dma_start(bias_t[:], bias_d[:])

            # PE pre-warm: HAM un-throttles (1.2 -> 2.4 GHz) only after
            # ~3.4us sustained activity; burn head DMA latency on dummies.
            warm_in = wmp.tile([128, 512], f16, tag="warm")
            nc.gpsimd.memset(warm_in[:], 0.0)
            warm_ps = psp.tile([128, 512], f32, tag="ps")
            for _ in range(12):
                nc.tensor.matmul(warm_ps[:], warm_in[:, :128], warm_in[:],
                                 start=True, stop=True)
            for _ in range(12):
                nc.tensor.matmul(warm_ps[:, :128], warm_in[:, :128],
                                 warm_in[:, :128], start=True, stop=True)

            # big region-tracked tiles; per-block DMAs keep deps fine-grained
            x16_t = xp16.tile([128, NUM_TIME_STEPS, BC], f16, tag="x16")
            x8_t = xp8.tile([128, NUM_TIME_STEPS, BC], f8, tag="x8")
            w16_t = wp16.tile([128, N_F16, 128], f16, tag="w16")
            w8_t = wp8.tile([128, N_PAIR, 2, 128], f8, tag="w8")

            # sync queue: the whole x16 stream, issued up-front BEFORE any
            # tile-inserted event waits land on sync, so every trigger
            # fires immediately and the queue streams at full rate.
            nc.sync.dma_start(x16_t[:, 0, :512], x16_d[:, 0, :512])
            nc.sync.dma_start(x16_t[:, 0, 512:], x16_d[:, 0, 512:])
            for j in range(1, NUM_TIME_STEPS, 2):
                je = min(j + 2, NUM_TIME_STEPS)
                nc.sync.dma_start(x16_t[:, j:je, :], x16_d[:, j:je, :])
            # scalar queue: tiny early w chunks; later only out triggers.
            nc.scalar.dma_start(w16_t[:, 0:4, :], w16_d[:, 0:4, :])
            nc.scalar.dma_start(w8_t[:, 0:1, :, :], w8_d[:, 0:1, :, :])

            # gpsimd queue (slow ~3us triggers): bulk weights, need-ordered
            for a, b in [(4, 24), (24, 64), (64, 112), (112, N_F16)]:
                nc.gpsimd.dma_start(w16_t[:, a:b, :], w16_d[:, a:b, :])
            for a, b in [(1, 8), (8, 16), (16, N_PAIR)]:
                nc.gpsimd.dma_start(w8_t[:, a:b, :, :], w8_d[:, a:b, :, :])

            # x8 derived on-device: e4m3(8 * x16) per block on DVE
            def convert(j):
                nc.vector.tensor_scalar_mul(x8_t[:, j, :], x16_t[:, j, :],
                                            8.0)

            convert(0)
            convert(1)

            inv = 1.0 / SCALE
            max_x8 = NUM_TIME_STEPS - TRI_BLOCK + 1  # highest x8 block read
            for o in range(NUM_TIME_STEPS):
                if 2 <= o + 1 <= max_x8:
                    convert(o + 1)  # x8[j] first read at o >= j+6
                lo = max(0, o - TRI_BLOCK + 1)
                f16bl = _f16_blocks(o)
                k0 = int(_K16[o])
                ps = [psp.tile([128, 512], f32, tag="ps", name=f"ps{o}_{h}")
                      for h in range(2)]
                started = [False, False]
                n_units = (1 if o >= 2 else 0) + len(f16bl)
                unit = 0
                if o >= 2:
                    unit += 1
                    for h in range(2):
                        nc.tensor.matmul(
                            ps[h][:],
                            w8_t[:, o - 2, :, :],
                            x8_t[:, lo:lo + 2, h * 512:(h + 1) * 512],
                            start=True, stop=(unit == n_units),
                            perf_mode=mybir.MatmulPerfMode.DoubleRow)
                    started = [True, True]
                for idx, j in enumerate(f16bl):
                    unit += 1
                    for h in range(2):
                        nc.tensor.matmul(
                            ps[h][:],
                            w16_t[:, k0 + idx, :],
                            x16_t[:, j, h * 512:(h + 1) * 512],
                            start=not started[h], stop=(unit == n_units))
                    started = [True, True]
                out_t = op.tile([128, BC], f16, tag="o")
                for h in range(2):
                    nc.vector.tensor_scalar(
                        out_t[:, h * 512:(h + 1) * 512], ps[h][:],
                        inv, bias_t[:, o:o + 1],
                        mybir.AluOpType.mult, mybir.AluOpType.add)
                nc.scalar.dma_start(yT_d[o, :, :], out_t[:])

    nc.compile()
    return nc


def _get_program():
    global _PROGRAM
    if _PROGRAM is None:
        _PROGRAM = _build_program()
    return _PROGRAM


def _pack_inputs(x, weight, bias, mask):
    x = np.asarray(x, dtype=np.float32)
    weight = np.asarray(weight, dtype=np.float32)
    bias = np.asarray(bias, dtype=np.float32)
    mask = np.asarray(mask)
    wm = weight * mask

    w16_flat = np.empty((128, N_F16 * 128), dtype=np.float16)
    k = 0
    for o in range(NUM_TIME_STEPS):
        for j in _f16_blocks(o):
            blk = wm[o * 128:(o + 1) * 128, j * 128:(j + 1) * 128]
            w16_flat[:, k * 128:(k + 1) * 128] = (blk.T * SCALE).astype(
                np.float16)
            k += 1

    w8_flat = np.empty((128, N_PAIR * 256), dtype=E4)
    for o in range(2, NUM_TIME_STEPS):
        lo = max(0, o - TRI_BLOCK + 1)
        p = o - 2
        for s, j in enumerate((lo, lo + 1)):
            blk = wm[o * 128:(o + 1) * 128, j * 128:(j + 1) * 128]
            w8_flat[:, p * 256 + s * 128:p * 256 + (s + 1) * 128] = (
                blk.T * SW).astype(E4)

    bias_t = np.ascontiguousarray(bias.reshape(NUM_TIME_STEPS, 128).T)

    x16 = x.astype(np.float16)
    in_maps = []
    for c in range(N_CORES):
        sl = slice(c * BC, (c + 1) * BC)
        x16c = np.ascontiguousarray(
            x16[sl].reshape(BC, NUM_TIME_STEPS, 128).transpose(2, 1, 0))
        in_maps.append({
            "x16": x16c,
            "w16": w16_flat.reshape(128, N_F16, 128),
            "w8": w8_flat.reshape(128, N_PAIR, 2, 128),
            "bias_t": bias_t,
        })
    return in_maps


def _run(inputs, trace=False):
    from concourse.bass_utils import run_bass_kernel_spmd

    nc = _get_program()
    in_maps = _pack_inputs(**inputs)
    res = run_bass_kernel_spmd(nc, in_maps, list(range(N_CORES)), trace=trace)

    y = np.empty((BATCH, OUT_SIZE), dtype=np.float32)
    for c in range(N_CORES):
        yTc = res.results[c]["yT"].astype(np.float32).reshape(OUT_SIZE, BC)
        y[c * BC:(c + 1) * BC] = yTc.T
    return y, res


def kernel(x, weight, bias, mask):
    y, _ = _run({"x": x, "weight": weight, "bias": bias, "mask": mask})
    return y
